# revision 1
# baseline (speedup 1.0000x reference)
"""Trainium2 Bass kernel for nn_EnhancedGAT (3-layer GATv2, N=10000, E=160000).

Strategy (8 NeuronCores, SPMD):
  - Destination-partition the graph: each core owns 1250 dst nodes (padded to
    1280 = 10 windows x 128). Edges (incl. self-loops) are sorted by dst on the
    host and bucketed into (core, window); each window's edge list is padded to
    T_w tiles of 128 edges.
  - Segment softmax without segment-max (logits are tiny):
        out = (sum_e exp(l_e) * xl[src_e]) / (sum_e exp(l_e) + eps)
    one pass over edges; scatter-add done with one-hot matmuls on the PE into
    PSUM accumulators per 128-dst window.
  - Layer 1 uses a "G-trick": aggregate w-weighted one-hot @ raw x features
    (128-dim) per window, then multiply by Wl1 once per window; avoids building
    or gathering the 1024-wide xl1 table entirely.
  - Layers 2/3 gather from xl tables built locally and AllGather'd across the
    8 cores (DRAM bounce + collective).
  - Gathers use the CounterMachine dma_gather (one descriptor batch per window
    instead of one SWDGE op per 128-row tile).
  - Each window runs in two passes to keep the PE stream free of the
    leaky/logits dependency chain: pass A computes edge scores (psum_S
    matmuls -> leaky -> fused mult+reduce -> exp -> stored w), pass B replays
    the one-hot scatter matmuls using stored w.
  - One-hot and weighted one-hot tiles are generated on-chip from dst indices
    (iota + is_equal [+ mult]), never streamed from HBM.
"""
import os
import numpy as np

import concourse.bass as bass
import concourse.bacc as bacc
import concourse.mybir as mybir
import concourse.tile as tile
from concourse.bass_utils import run_bass_kernel_spmd
from concourse.masks import make_identity

F32 = mybir.dt.float32
I32 = mybir.dt.int32
I16 = mybir.dt.int16
AF = mybir.ActivationFunctionType
OP = mybir.AluOpType

NC_CORES = 8
N = 10000
ND = 128
ED = 32
PER = N // NC_CORES          # 1250
NPAD = 1280
W = NPAD // 128              # 10 windows
EPS = 1e-16

_DEBUG = bool(int(os.environ.get("GAT_DEBUG", "0")))
_WLIM = int(os.environ.get("GAT_WLIM", str(W)))    # windows to emit (dev only)
_LAYERS = int(os.environ.get("GAT_LAYERS", "3"))   # dev only
_NOCOLL = bool(int(os.environ.get("GAT_NOCOLL", "0")))  # sim only
_REPEAT = int(os.environ.get("GAT_REPEAT", "1"))  # timing: run whole pipeline R times
_NODG = bool(int(os.environ.get("GAT_NODG", "0")))  # fallback: per-tile indirect gathers
_NOTTR = bool(int(os.environ.get("GAT_NOTTR", "0")))  # fallback: mult+reduce logits


# ----------------------------------------------------------------------------
# host-side prep
# ----------------------------------------------------------------------------

def _host_prep(x, edge_index, edge_attr):
    src = np.concatenate([edge_index[0], np.arange(N)]).astype(np.int64)
    dst = np.concatenate([edge_index[1], np.arange(N)]).astype(np.int64)
    ea = np.concatenate(
        [edge_attr, np.tile(edge_attr.mean(0), (N, 1))], axis=0
    ).astype(np.float32)

    core_of = dst // PER
    loc = dst % PER
    win_of = loc // 128
    dst_rel = (loc % 128).astype(np.float32)

    key = core_of * W + win_of
    order = np.argsort(key, kind="stable")
    counts = np.bincount(key[order], minlength=NC_CORES * W)
    T_w = int(np.ceil(counts.max() / 128))
    EPW = T_w * 128

    starts = np.zeros(NC_CORES * W, np.int64)
    starts[1:] = np.cumsum(counts)[:-1]

    src1 = np.zeros((NC_CORES, W, EPW), np.int32)
    src23 = np.zeros((NC_CORES, W, EPW), np.int32)
    drel = np.full((NC_CORES, W, EPW), -1.0, np.float32)
    eaT = np.zeros((NC_CORES, W, ED, EPW), np.float32)
    for c in range(NC_CORES):
        for w in range(W):
            k = int(counts[c * W + w])
            m = order[starts[c * W + w]: starts[c * W + w] + k]
            src1[c, w, :k] = src[m]
            src23[c, w, :k] = (src[m] // PER) * NPAD + (src[m] % PER)
            drel[c, w, :k] = dst_rel[m]
            eaT[c, w, :, :k] = ea[m].T
    return T_w, EPW, src1, src23, drel, eaT


def _pad_own(a, c):
    out = np.zeros((NPAD,) + a.shape[1:], a.dtype)
    out[:PER] = a[c * PER: (c + 1) * PER]
    return out


def _chunks_for_rhs(Wm):
    """[K, F] weight -> [128, (K//128)*F]: chunk k at cols [k*F:(k+1)*F]."""
    K, F = Wm.shape
    assert K % 128 == 0
    return np.ascontiguousarray(
        Wm.reshape(K // 128, 128, F).transpose(1, 0, 2).reshape(128, -1)
    )


def _idx16_wrap(idx, EPW):
    """[W, EPW] int -> [W*128, EPW//16] int16 in dma_gather layout:
    index for gathered row i of window w at [w*128 + i%16, i//16], with the
    16-partition wrap replicated across all 8 GpSimd core banks (128 rows)."""
    Wn = idx.shape[0]
    wrap = idx.reshape(Wn, EPW // 16, 16).transpose(0, 2, 1)   # [W, 16, EPW//16]
    out = np.tile(wrap, (1, 8, 1)).astype(np.int16)            # [W, 128, EPW//16]
    return np.ascontiguousarray(out.reshape(Wn * 128, EPW // 16))


# ----------------------------------------------------------------------------
# bass program
# ----------------------------------------------------------------------------

def _build_program(T_w, repeat=None):
    repeat = _REPEAT if repeat is None else repeat
    EPW = T_w * 128
    EPW16 = EPW // 16
    nc = bacc.Bacc("TRN2", target_bir_lowering=False, debug=False,
                   enable_asserts=False, num_devices=NC_CORES)

    def din(name, shape, dt=F32):
        return nc.dram_tensor(name, shape, dt, kind="ExternalInput")

    x_full = din("x_full", [N, ND])
    x_ownT = din("x_ownT", [ND, NPAD])
    src1_d = din("src1i", [W * 128, T_w], I32)
    src23_d = din("src23i", [W * 128, T_w], I32)
    drel_d = din("drel", [W * 128, T_w])
    eaT_d = din("eaT", [W * ED, EPW])
    iotar_d = din("iotar", [128, 128])

    Wl1_d = din("Wl1", [128, 1024])
    Wr1_d = din("Wr1", [128, 1024])
    Wres_d = din("Wres", [128, 1024])
    We1_d = din("We1", [ED, 1024])
    att1_d = din("att1b", [128, 1024])
    Wl2_d = din("Wl2c", [128, 8 * 512])
    Wr2_d = din("Wr2c", [128, 8 * 512])
    We2_d = din("We2", [ED, 512])
    att2_d = din("att2b", [128, 512])
    Wl3_d = din("Wl3", [128, 128])
    Wr3_d = din("Wr3", [128, 128])
    We3_d = din("We3", [ED, 128])
    att3_d = din("att3b", [128, 128])
    Wc1_d = din("Wc1", [128, 64])
    Wc2_d = din("Wc2", [64, 3])
    biasr1_d = din("biasr1", [1, 1024])
    const1_d = din("const1", [1, 1024])
    biasr2_d = din("biasr2", [1, 512])
    const2_d = din("const2b", [128, 128])
    biasr3_d = din("biasr3", [1, 128])
    const3_d = din("const3b", [128, 128])
    bc1_d = din("bc1", [1, 64])
    bc2_d = din("bc2", [1, 3])

    out_d = nc.dram_tensor("out_o", [NPAD, 3], F32, kind="ExternalOutput")
    if _DEBUG:
        h1_dbg = nc.dram_tensor("h1_dbg", [NPAD, 1024], F32, kind="ExternalOutput")
        h2_dbg = nc.dram_tensor("h2_dbg", [NPAD, 128], F32, kind="ExternalOutput")
        h3_dbg = nc.dram_tensor("h3_dbg", [NPAD, 128], F32, kind="ExternalOutput")

    with tile.TileContext(nc) as tc:
        with tc.tile_pool(name="wp", bufs=1) as wp, \
             tc.tile_pool(name="slab", bufs=1) as slab, \
             tc.tile_pool(name="io", bufs=2) as io, \
             tc.tile_pool(name="io3", bufs=3) as io3, \
             tc.tile_pool(name="fat", bufs=2) as fat, \
             tc.tile_pool(name="tp1", bufs=1) as tp1, \
             tc.tile_pool(name="big", bufs=2) as big, \
             tc.tile_pool(name="psS", bufs=2, space="PSUM") as psS, \
             tc.tile_pool(name="psG", bufs=1, space="PSUM") as psG, \
             tc.tile_pool(name="psT", bufs=1, space="PSUM") as psT, \
             tc.tile_pool(name="dram", bufs=1, space="DRAM") as dr:

            # ---------- resident constants ----------
            def load(dram_t, shape, name, dt=F32):
                t = wp.tile(shape, dt, name=name, tag=name)
                nc.sync.dma_start(out=t[:], in_=dram_t.ap())
                return t

            Wl1 = load(Wl1_d, [128, 1024], "Wl1")
            Wr1 = load(Wr1_d, [128, 1024], "Wr1")
            Wres = load(Wres_d, [128, 1024], "Wres")
            We1 = load(We1_d, [ED, 1024], "We1")
            att1b = load(att1_d, [128, 1024], "att1b")
            We2 = load(We2_d, [ED, 512], "We2")
            att2b = load(att2_d, [128, 512], "att2b")
            Wl3 = load(Wl3_d, [128, 128], "Wl3")
            Wr3 = load(Wr3_d, [128, 128], "Wr3")
            We3 = load(We3_d, [ED, 128], "We3")
            att3b = load(att3_d, [128, 128], "att3b")
            Wc1 = load(Wc1_d, [128, 64], "Wc1")
            Wc2 = load(Wc2_d, [64, 3], "Wc2")
            biasr1 = load(biasr1_d, [1, 1024], "biasr1")
            const1 = load(const1_d, [1, 1024], "const1")
            biasr2 = load(biasr2_d, [1, 512], "biasr2")
            const2b = load(const2_d, [128, 128], "const2b")
            biasr3 = load(biasr3_d, [1, 128], "biasr3")
            const3b = load(const3_d, [128, 128], "const3b")
            bc1 = load(bc1_d, [1, 64], "bc1")
            bc2 = load(bc2_d, [1, 3], "bc2")
            iotar = load(iotar_d, [128, 128], "iotar")

            ident = wp.tile([128, 128], F32, name="ident", tag="ident")
            make_identity(nc, ident[:])
            ones1 = wp.tile([1, 128], F32, name="ones1", tag="ones1")
            nc.vector.memset(ones1[:], 1.0)

            xr2_own = slab.tile([128, W * 512], F32, name="xr2_own", tag="xr2_own")
            xr3_own = slab.tile([128, W * 128], F32, name="xr3_own", tag="xr3_own")


            # ---------- helpers ----------
            def window_meta(w, lidx):
                idx_w = io.tile([128, T_w], I32, name=f"idx{lidx}_{w}", tag="idx")
                src_d = src1_d if lidx == 1 else src23_d
                nc.sync.dma_start(out=idx_w[:], in_=src_d.ap()[w * 128:(w + 1) * 128, :])
                drel_w = io.tile([128, T_w], F32, name=f"drel{lidx}_{w}", tag="drel")
                nc.sync.dma_start(out=drel_w[:], in_=drel_d.ap()[w * 128:(w + 1) * 128, :])
                ea_w = big.tile([ED, EPW], F32, name=f"ea{lidx}_{w}", tag="ea")
                nc.sync.dma_start(out=ea_w[:], in_=eaT_d.ap()[w * ED:(w + 1) * ED, :])
                return idx_w, drel_w, ea_w

            def gather_rows(idx_w, table_ap, Fdim, pool, tag, name, col0=0, nrows=None):
                """Batched row gather: ONE SWDGE indirect DMA for nrows rows
                (idx AP [128, nrows/128]; out[p, t*F:(t+1)*F] = table[idx[p,t]],
                i.e. edge t*128+p of the window -- tile-major)."""
                nrows = EPW if nrows is None else nrows
                nt = nrows // 128
                t0 = col0 // 128
                g = pool.tile([128, nt * Fdim], F32, name=name, tag=tag)
                for tt in range(nt):
                    nc.gpsimd.indirect_dma_start(
                        out=g[:, tt * Fdim:(tt + 1) * Fdim], out_offset=None,
                        in_=table_ap,
                        in_offset=bass.IndirectOffsetOnAxis(
                            ap=idx_w[:, t0 + tt:t0 + tt + 1], axis=0))
                return g

            def onehot(drel_w, t, w, lidx, sfx=""):
                oh = io3.tile([128, 128], F32, name=f"oh{lidx}_{w}_{t}{sfx}", tag="oh")
                nc.vector.tensor_scalar(out=oh[:], in0=iotar[:],
                                        scalar1=drel_w[:, t:t + 1], scalar2=None,
                                        op0=OP.is_equal)
                return oh

            def leaky(pt, F, name, pool):
                """s = max(v, 0.2 v); v may be PSUM or SBUF AP."""
                s = pool.tile([128, F], F32, name=name, tag=f"lk{F}")
                nc.scalar.activation(out=s[:], in_=pt, func=AF.Copy, scale=0.2)
                nc.vector.tensor_tensor(out=s[:], in0=pt, in1=s[:], op=OP.max)
                return s

            def wexp_of(s, attb, H, we_out, w, t, lidx):
                """w = exp(per-head <s, att>) via fused mult+reduce, into we_out."""
                lg = io.tile([128, H], F32, name=f"lg{lidx}_{w}_{t}", tag="lg")
                nc.vector.tensor_tensor(out=s[:], in0=s[:], in1=attb[:, :H * 128],
                                        op=OP.mult)
                uv = s[:].rearrange("p (h c) -> p h c", h=H) if H > 1 else s[:]
                nc.vector.tensor_reduce(out=lg[:], in_=uv,
                                        axis=mybir.AxisListType.X, op=OP.add)
                nc.scalar.activation(out=we_out, in_=lg[:], func=AF.Exp)

            def rz_from(ps_z, H, w, lidx, quarter=False):
                zt = io.tile([128, H], F32, name=f"zt{lidx}_{w}", tag="zt")
                nc.vector.tensor_scalar(out=zt[:], in0=ps_z, scalar1=EPS,
                                        scalar2=None, op0=OP.add)
                rz = io.tile([128, H], F32, name=f"rz{lidx}_{w}", tag="rz")
                nc.vector.reciprocal(out=rz[:], in_=zt[:])
                if quarter:
                    nc.vector.tensor_scalar(out=rz[:], in0=rz[:], scalar1=0.25,
                                            scalar2=None, op0=OP.mult)
                return rz

            def elu_of(a, F, w, lidx):
                """h = elu(a) = relu(a) + min(exp(a)-1, 0)."""
                ex = io.tile([128, F], F32, name=f"ex{lidx}_{w}", tag="ex")
                nc.scalar.activation(out=ex[:], in_=a[:], func=AF.Exp)
                em = io.tile([128, F], F32, name=f"em{lidx}_{w}", tag="em")
                nc.vector.tensor_scalar(out=em[:], in0=ex[:], scalar1=1.0, scalar2=0.0,
                                        op0=OP.subtract, op1=OP.min)
                r = io.tile([128, F], F32, name=f"r{lidx}_{w}", tag="r")
                nc.scalar.activation(out=r[:], in_=a[:], func=AF.Relu)
                h = io.tile([128, F], F32, name=f"h{lidx}_{w}", tag="helu")
                nc.vector.tensor_tensor(out=h[:], in0=r[:], in1=em[:], op=OP.add)
                return h

            def transpose_to(sb_in, F, w, lidx, tag):
                tout = tp1.tile([128, F], F32, name=f"T{tag}{lidx}_{w}", tag="Ttmp")
                for k in range(F // 128):
                    ps = psT.tile([128, 128], F32, name=f"ps{tag}{lidx}_{w}_{k}", tag="scr")
                    nc.tensor.transpose(out=ps[:], in_=sb_in[:, k * 128:(k + 1) * 128],
                                        identity=ident[:])
                    nc.scalar.copy(out=tout[:, k * 128:(k + 1) * 128], in_=ps[:])
                return tout

            def ohw_tile(drel_w, we_w, H, t, h, out_sl, engine):
                """weighted one-hot: out[e,d] = (d==drel[e]) * w_h[e] in one op."""
                engine.tensor_scalar(
                    out=out_sl, in0=iotar[:],
                    scalar1=drel_w[:, t:t + 1],
                    scalar2=we_w[:, t * H + h:t * H + h + 1],
                    op0=OP.is_equal, op1=OP.mult)

            for rep in range(repeat):
                xl2_bounce = dr.tile([NPAD, 512], F32, name=f"xl2_bounce_{rep}",
                                     tag=f"x2b{rep}")
                xl2_full = dr.tile([NC_CORES * NPAD, 512], F32, name=f"xl2_full_{rep}",
                                   addr_space="Shared", tag=f"x2f{rep}")
                xl3_bounce = dr.tile([NPAD, 128], F32, name=f"xl3_bounce_{rep}",
                                     tag=f"x3b{rep}")
                xl3_full = dr.tile([NC_CORES * NPAD, 128], F32, name=f"xl3_full_{rep}",
                                   addr_space="Shared", tag=f"x3f{rep}")
                # =========================================================
                # LAYER 1 (H=8, G-trick) + xl2/xr2 build
                # =========================================================
                for w in range(_WLIM):
                    idx_w, drel_w, ea_w = window_meta(w, 1)
                    xo_w = io.tile([128, 128], F32, name=f"xo_{w}", tag="xo")
                    nc.sync.dma_start(out=xo_w[:], in_=x_ownT.ap()[:, w * 128:(w + 1) * 128])
                    xga = gather_rows(idx_w, x_full.ap(), 128, big, "gxa", f"gxa1_{w}")
                    we_w = io.tile([128, T_w * 8], F32, name=f"wew1_{w}", tag="wew")

                    # xr1 window: x_win @ Wr1 + (br1+bl1)
                    ps_xr = psS.tile([128, 1024], F32, name=f"psxr1_{w}", tag="S")
                    for j in range(2):
                        sl = slice(j * 512, (j + 1) * 512)
                        nc.tensor.matmul(out=ps_xr[:, sl], lhsT=xo_w[:],
                                         rhs=Wr1[:, sl], start=True, stop=False)
                        nc.tensor.matmul(out=ps_xr[:, sl], lhsT=ones1[:],
                                         rhs=biasr1[:, sl], start=False, stop=True)
                    xr1_w = big.tile([128, 1024], F32, name=f"xr1_{w}", tag="xr1")
                    nc.scalar.copy(out=xr1_w[:], in_=ps_xr[:])

                    # ---- pass A: edge scores ----
                    for t in range(T_w):
                        oh = onehot(drel_w, t, w, 1)
                        ohT_ps = psT.tile([128, 128], F32, name=f"pso1_{w}_{t}", tag="scr")
                        nc.tensor.transpose(out=ohT_ps[:], in_=oh[:], identity=ident[:])
                        ohT = io3.tile([128, 128], F32, name=f"ohT1_{w}_{t}", tag="ohT")
                        nc.scalar.copy(out=ohT[:], in_=ohT_ps[:])
                        psx = psT.tile([128, 128], F32, name=f"psx1_{w}_{t}", tag="scr")
                        nc.tensor.transpose(out=psx[:], in_=xga[:, t * 128:(t + 1) * 128],
                                            identity=ident[:])
                        xgT = io3.tile([128, 128], F32, name=f"xgT1_{w}_{t}", tag="xgT1")
                        nc.scalar.copy(out=xgT[:], in_=psx[:])

                        ps_S = psS.tile([128, 1024], F32, name=f"psS1_{w}_{t}", tag="S")
                        for j in range(2):
                            sl = slice(j * 512, (j + 1) * 512)
                            nc.tensor.matmul(out=ps_S[:, sl], lhsT=xgT[:],
                                             rhs=Wl1[:, sl], start=True, stop=False)
                        for j in range(2):
                            sl = slice(j * 512, (j + 1) * 512)
                            nc.tensor.matmul(out=ps_S[:, sl],
                                             lhsT=ea_w[:, t * 128:(t + 1) * 128],
                                             rhs=We1[:, sl], start=False, stop=False)
                        for j in range(2):
                            sl = slice(j * 512, (j + 1) * 512)
                            nc.tensor.matmul(out=ps_S[:, sl], lhsT=ohT[:],
                                             rhs=xr1_w[:, sl], start=False, stop=True)
                        s = leaky(ps_S[:], 1024, f"s1_{w}_{t}", fat)
                        wexp_of(s, att1b, 8, we_w[:, t * 8:(t + 1) * 8], w, t, 1)

                    # ---- pass B: weighted scatter (G-trick) ----
                    # rhs_ext[e, h*128+k] = w_h[e] * xg[e, k]; cols 1024:1032 = w.
                    # One stationary one-hot per tile, 3 matmuls (512+512+8).
                    ps_G = psG.tile([128, 1032], F32, name=f"psG_{w}", tag="G")
                    for t in range(T_w):
                        oh = onehot(drel_w, t, w, 1, sfx="b")
                        re = fat.tile([128, 1032], F32, name=f"re_{w}_{t}", tag="ohw")
                        for h in range(8):
                            xsl = xga[:, t * 128:(t + 1) * 128]
                            osl = re[:, h * 128:(h + 1) * 128]
                            wsl = we_w[:, t * 8 + h:t * 8 + h + 1]
                            if h % 2 == 0:
                                nc.scalar.activation(out=osl, in_=xsl, func=AF.Copy,
                                                     scale=wsl)
                            else:
                                nc.vector.tensor_scalar(out=osl, in0=xsl, scalar1=wsl,
                                                        scalar2=None, op0=OP.mult)
                        nc.vector.tensor_copy(out=re[:, 1024:1032],
                                              in_=we_w[:, t * 8:(t + 1) * 8])
                        for j, (c0, c1) in enumerate(((0, 512), (512, 1024), (1024, 1032))):
                            nc.tensor.matmul(out=ps_G[:, c0:c1], lhsT=oh[:],
                                             rhs=re[:, c0:c1],
                                             start=(t == 0), stop=(t == T_w - 1))

                    # ---- window flush ----
                    rz = rz_from(ps_G[:, 1024:1032], 8, w, 1)
                    gn = fat.tile([128, 1024], F32, name=f"gn_{w}", tag="lk1024")
                    for h in range(8):
                        nc.scalar.activation(out=gn[:, h * 128:(h + 1) * 128],
                                             in_=ps_G[:, h * 128:(h + 1) * 128],
                                             func=AF.Copy, scale=rz[:, h:h + 1])
                    gnT = transpose_to(gn, 1024, w, 1, "g")
                    ps_O = psS.tile([128, 1024], F32, name=f"psO1_{w}", tag="S")
                    for h in range(8):
                        nc.tensor.matmul(out=ps_O[:, h * 128:(h + 1) * 128],
                                         lhsT=gnT[:, h * 128:(h + 1) * 128],
                                         rhs=Wl1[:, h * 128:(h + 1) * 128],
                                         start=(h % 4 == 0), stop=False)
                    for j in range(2):
                        sl = slice(j * 512, (j + 1) * 512)
                        nc.tensor.matmul(out=ps_O[:, sl], lhsT=xo_w[:],
                                         rhs=Wres[:, sl], start=False, stop=False)
                        nc.tensor.matmul(out=ps_O[:, sl], lhsT=ones1[:],
                                         rhs=const1[:, sl], start=False, stop=True)
                    h1_w = leaky(ps_O[:], 1024, f"h1_{w}", fat)
                    if _DEBUG:
                        nc.sync.dma_start(out=h1_dbg.ap()[w * 128:(w + 1) * 128, :],
                                          in_=h1_w[:])
                    h1T = transpose_to(h1_w, 1024, w, 1, "h")
                    Wl2 = tp1.tile([128, 8 * 512], F32, name=f"Wl2s_{w}", tag="w2s")
                    nc.sync.dma_start(out=Wl2[:], in_=Wl2_d.ap())
                    ps_x2 = psS.tile([128, 512], F32, name=f"psx2_{w}", tag="S")
                    for k in range(8):
                        nc.tensor.matmul(out=ps_x2[:], lhsT=h1T[:, k * 128:(k + 1) * 128],
                                         rhs=Wl2[:, k * 512:(k + 1) * 512],
                                         start=(k == 0), stop=(k == 7))
                    xl2_w = io.tile([128, 512], F32, name=f"xl2_{w}", tag="xl2")
                    nc.scalar.copy(out=xl2_w[:], in_=ps_x2[:])
                    nc.sync.dma_start(out=xl2_bounce[w * 128:(w + 1) * 128, :], in_=xl2_w[:])
                    Wr2 = tp1.tile([128, 8 * 512], F32, name=f"Wr2s_{w}", tag="w2s")
                    nc.sync.dma_start(out=Wr2[:], in_=Wr2_d.ap())
                    ps_r2 = psS.tile([128, 512], F32, name=f"psr2_{w}", tag="S")
                    for k in range(8):
                        nc.tensor.matmul(out=ps_r2[:], lhsT=h1T[:, k * 128:(k + 1) * 128],
                                         rhs=Wr2[:, k * 512:(k + 1) * 512],
                                         start=(k == 0), stop=False)
                    nc.tensor.matmul(out=ps_r2[:], lhsT=ones1[:], rhs=biasr2[:],
                                     start=False, stop=True)
                    nc.scalar.copy(out=xr2_own[:, w * 512:(w + 1) * 512], in_=ps_r2[:])

                # =========================================================
                # LAYER 2 (H=4, gather xl2)
                # =========================================================
                if _LAYERS >= 2:
                    if _NOCOLL:
                        nc.sync.dma_start(out=xl2_full[0:NPAD, :], in_=xl2_bounce[:])
                    else:
                        nc.gpsimd.collective_compute(
                            "AllGather", OP.bypass,
                            replica_groups=[list(range(NC_CORES))],
                            ins=[xl2_bounce[:]], outs=[xl2_full[:]])

                    GS = 2  # tiles per gather group (512-wide rows)
                    ngrp = (T_w + GS - 1) // GS
                    for w in range(_WLIM):
                        idx_w, drel_w, ea_w = window_meta(w, 2)
                        we_w = io.tile([128, T_w * 4], F32, name=f"wew2_{w}", tag="wew")

                        # ---- pass A ----
                        for g in range(ngrp):
                            gs = min(GS, T_w - g * GS)
                            xlgA = gather_rows(idx_w, xl2_full[:], 512, fat, "g2a",
                                               f"g2a_{w}_{g}", col0=g * GS * 128,
                                               nrows=gs * 128)
                            for tt in range(gs):
                                t = g * GS + tt
                                oh = onehot(drel_w, t, w, 2)
                                ohT_ps = psT.tile([128, 128], F32, name=f"pso2_{w}_{t}", tag="scr")
                                nc.tensor.transpose(out=ohT_ps[:], in_=oh[:], identity=ident[:])
                                ohT = io3.tile([128, 128], F32, name=f"ohT2_{w}_{t}", tag="ohT")
                                nc.scalar.copy(out=ohT[:], in_=ohT_ps[:])
                                ps_B = psS.tile([128, 512], F32, name=f"psB2_{w}_{t}", tag="S")
                                nc.tensor.matmul(out=ps_B[:],
                                                 lhsT=ea_w[:, t * 128:(t + 1) * 128],
                                                 rhs=We2[:], start=True, stop=False)
                                nc.tensor.matmul(out=ps_B[:], lhsT=ohT[:],
                                                 rhs=xr2_own[:, w * 512:(w + 1) * 512],
                                                 start=False, stop=True)
                                s = fat.tile([128, 512], F32, name=f"s2_{w}_{t}", tag="lk512")
                                nc.vector.tensor_tensor(out=s[:],
                                                        in0=xlgA[:, tt * 512:(tt + 1) * 512],
                                                        in1=ps_B[:], op=OP.add)
                                t02 = io.tile([128, 512], F32, name=f"t022_{w}_{t}", tag="t02b")
                                nc.scalar.activation(out=t02[:], in_=s[:], func=AF.Copy,
                                                     scale=0.2)
                                nc.vector.tensor_tensor(out=s[:], in0=s[:], in1=t02[:],
                                                        op=OP.max)
                                wexp_of(s, att2b, 4, we_w[:, t * 4:(t + 1) * 4], w, t, 2)

                        # ---- pass B ----
                        ps_O2 = psG.tile([128, 516], F32, name=f"psO2_{w}", tag="G")
                        for g in range(ngrp):
                            gs = min(GS, T_w - g * GS)
                            xlgB = gather_rows(idx_w, xl2_full[:], 512, fat, "g2b",
                                               f"g2b_{w}_{g}", col0=g * GS * 128,
                                               nrows=gs * 128)
                            for tt in range(gs):
                                t = g * GS + tt
                                oh = onehot(drel_w, t, w, 2, sfx="b")
                                At = io.tile([128, 516], F32, name=f"At2_{w}_{t}", tag="At2")
                                for h in range(4):
                                    src_sl = xlgB[:, tt * 512 + h * 128: tt * 512 + (h + 1) * 128]
                                    if h % 2 == 0:
                                        nc.scalar.activation(
                                            out=At[:, h * 128:(h + 1) * 128], in_=src_sl,
                                            func=AF.Copy,
                                            scale=we_w[:, t * 4 + h:t * 4 + h + 1])
                                    else:
                                        nc.vector.tensor_scalar(
                                            out=At[:, h * 128:(h + 1) * 128], in0=src_sl,
                                            scalar1=we_w[:, t * 4 + h:t * 4 + h + 1],
                                            scalar2=None, op0=OP.mult)
                                nc.vector.tensor_copy(out=At[:, 512:516],
                                                      in_=we_w[:, t * 4:(t + 1) * 4])
                                nc.tensor.matmul(out=ps_O2[:, 0:512], lhsT=oh[:],
                                                 rhs=At[:, 0:512],
                                                 start=(t == 0), stop=(t == T_w - 1))
                                nc.tensor.matmul(out=ps_O2[:, 512:516], lhsT=oh[:],
                                                 rhs=At[:, 512:516],
                                                 start=(t == 0), stop=(t == T_w - 1))

                        # ---- flush: mean over heads + elu + xl3/xr3 builds ----
                        rz = rz_from(ps_O2[:, 512:516], 4, w, 2, quarter=True)
                        m4 = io.tile([128, 512], F32, name=f"m4_{w}", tag="m4")
                        for h in range(4):
                            nc.scalar.activation(out=m4[:, h * 128:(h + 1) * 128],
                                                 in_=ps_O2[:, h * 128:(h + 1) * 128],
                                                 func=AF.Copy, scale=rz[:, h:h + 1])
                        m01 = io.tile([128, 128], F32, name=f"m01_{w}", tag="m01")
                        nc.vector.tensor_tensor(out=m01[:], in0=m4[:, 0:128],
                                                in1=m4[:, 128:256], op=OP.add)
                        m23 = io.tile([128, 128], F32, name=f"m23_{w}", tag="m23")
                        nc.vector.tensor_tensor(out=m23[:], in0=m4[:, 256:384],
                                                in1=m4[:, 384:512], op=OP.add)
                        a2 = io.tile([128, 128], F32, name=f"a2_{w}", tag="a2")
                        nc.vector.tensor_tensor(out=a2[:], in0=m01[:], in1=m23[:], op=OP.add)
                        nc.vector.tensor_tensor(out=a2[:], in0=a2[:], in1=const2b[:], op=OP.add)
                        h2_w = elu_of(a2, 128, w, 2)
                        if _DEBUG:
                            nc.sync.dma_start(out=h2_dbg.ap()[w * 128:(w + 1) * 128, :],
                                              in_=h2_w[:])
                        h2T = transpose_to(h2_w, 128, w, 2, "h2")
                        ps_x3 = psT.tile([128, 128], F32, name=f"psx3_{w}", tag="scr")
                        nc.tensor.matmul(out=ps_x3[:], lhsT=h2T[:], rhs=Wl3[:],
                                         start=True, stop=True)
                        xl3_w = io.tile([128, 128], F32, name=f"xl3_{w}", tag="xl3")
                        nc.scalar.copy(out=xl3_w[:], in_=ps_x3[:])
                        nc.sync.dma_start(out=xl3_bounce[w * 128:(w + 1) * 128, :], in_=xl3_w[:])
                        ps_r3 = psT.tile([128, 128], F32, name=f"psr3_{w}", tag="scr")
                        nc.tensor.matmul(out=ps_r3[:], lhsT=h2T[:], rhs=Wr3[:],
                                         start=True, stop=False)
                        nc.tensor.matmul(out=ps_r3[:], lhsT=ones1[:], rhs=biasr3[:],
                                         start=False, stop=True)
                        nc.scalar.copy(out=xr3_own[:, w * 128:(w + 1) * 128], in_=ps_r3[:])

                # =========================================================
                # LAYER 3 (H=1) + head
                # =========================================================
                if _LAYERS >= 3:
                    if _NOCOLL:
                        nc.sync.dma_start(out=xl3_full[0:NPAD, :], in_=xl3_bounce[:])
                    else:
                        nc.gpsimd.collective_compute(
                            "AllGather", OP.bypass,
                            replica_groups=[list(range(NC_CORES))],
                            ins=[xl3_bounce[:]], outs=[xl3_full[:]])

                    for w in range(_WLIM):
                        idx_w, drel_w, ea_w = window_meta(w, 3)
                        xga3 = gather_rows(idx_w, xl3_full[:], 128, big, "gxa", f"gxa3_{w}")
                        we_w = io.tile([128, T_w], F32, name=f"wew3_{w}", tag="wew")

                        # ---- pass A ----
                        for t in range(T_w):
                            oh = onehot(drel_w, t, w, 3)
                            ohT_ps = psT.tile([128, 128], F32, name=f"pso3_{w}_{t}", tag="scr")
                            nc.tensor.transpose(out=ohT_ps[:], in_=oh[:], identity=ident[:])
                            ohT = io3.tile([128, 128], F32, name=f"ohT3_{w}_{t}", tag="ohT")
                            nc.scalar.copy(out=ohT[:], in_=ohT_ps[:])
                            ps_B = psS.tile([128, 128], F32, name=f"psB3_{w}_{t}", tag="S")
                            nc.tensor.matmul(out=ps_B[:], lhsT=ea_w[:, t * 128:(t + 1) * 128],
                                             rhs=We3[:], start=True, stop=False)
                            nc.tensor.matmul(out=ps_B[:], lhsT=ohT[:],
                                             rhs=xr3_own[:, w * 128:(w + 1) * 128],
                                             start=False, stop=True)
                            s = fat.tile([128, 128], F32, name=f"s3_{w}_{t}", tag="lk128")
                            nc.vector.tensor_tensor(out=s[:],
                                                    in0=xga3[:, t * 128:(t + 1) * 128],
                                                    in1=ps_B[:], op=OP.add)
                            t02 = io.tile([128, 128], F32, name=f"t023_{w}_{t}", tag="t02c")
                            nc.scalar.activation(out=t02[:], in_=s[:], func=AF.Copy, scale=0.2)
                            nc.vector.tensor_tensor(out=s[:], in0=s[:], in1=t02[:], op=OP.max)
                            wexp_of(s, att3b, 1, we_w[:, t:t + 1], w, t, 3)

                        # ---- pass B ----
                        ps_O3 = psG.tile([128, 129], F32, name=f"psO3_{w}", tag="G")
                        for t in range(T_w):
                            oh = onehot(drel_w, t, w, 3, sfx="b")
                            At = io3.tile([128, 129], F32, name=f"At3_{w}_{t}", tag="At3")
                            nc.scalar.activation(out=At[:, 0:128],
                                                 in_=xga3[:, t * 128:(t + 1) * 128],
                                                 func=AF.Copy, scale=we_w[:, t:t + 1])
                            nc.vector.tensor_copy(out=At[:, 128:129], in_=we_w[:, t:t + 1])
                            nc.tensor.matmul(out=ps_O3[:], lhsT=oh[:], rhs=At[:],
                                             start=(t == 0), stop=(t == T_w - 1))

                        # ---- flush + head ----
                        rz = rz_from(ps_O3[:, 128:129], 1, w, 3)
                        o3 = io.tile([128, 128], F32, name=f"o3_{w}", tag="o3")
                        nc.scalar.activation(out=o3[:], in_=ps_O3[:, 0:128],
                                             func=AF.Copy, scale=rz[:, 0:1])
                        nc.vector.tensor_tensor(out=o3[:], in0=o3[:], in1=const3b[:], op=OP.add)
                        h3_w = elu_of(o3, 128, w, 3)
                        if _DEBUG:
                            nc.sync.dma_start(out=h3_dbg.ap()[w * 128:(w + 1) * 128, :],
                                              in_=h3_w[:])
                        h3T = transpose_to(h3_w, 128, w, 3, "h3")
                        ps_c1 = psT.tile([128, 64], F32, name=f"psc1_{w}", tag="scr")
                        nc.tensor.matmul(out=ps_c1[:], lhsT=h3T[:], rhs=Wc1[:],
                                         start=True, stop=False)
                        nc.tensor.matmul(out=ps_c1[:], lhsT=ones1[:], rhs=bc1[:],
                                         start=False, stop=True)
                        a1 = io.tile([128, 64], F32, name=f"a1_{w}", tag="a1")
                        nc.scalar.copy(out=a1[:], in_=ps_c1[:])
                        c1 = elu_of(a1, 64, w, 4)
                        ps_t = psT.tile([128, 128], F32, name=f"psct_{w}", tag="scr")
                        nc.tensor.transpose(out=ps_t[0:64, :], in_=c1[:], identity=ident[:])
                        c1T = io.tile([64, 128], F32, name=f"c1T_{w}", tag="c1T")
                        nc.scalar.copy(out=c1T[:], in_=ps_t[0:64, :])
                        ps_f = psT.tile([128, 3], F32, name=f"psf_{w}", tag="scr")
                        nc.tensor.matmul(out=ps_f[:], lhsT=c1T[:], rhs=Wc2[:],
                                         start=True, stop=False)
                        nc.tensor.matmul(out=ps_f[:], lhsT=ones1[:], rhs=bc2[:],
                                         start=False, stop=True)
                        fo = io.tile([128, 3], F32, name=f"fo_{w}", tag="fo")
                        nc.scalar.copy(out=fo[:], in_=ps_f[:])
                        nc.sync.dma_start(out=out_d.ap()[w * 128:(w + 1) * 128, :], in_=fo[:])

    nc.compile()
    return nc


# ----------------------------------------------------------------------------
# entry point
# ----------------------------------------------------------------------------

_cache = {}
_last_in_maps = None


def kernel(**inputs):
    x = np.ascontiguousarray(np.asarray(inputs["x"], dtype=np.float32))
    edge_index = np.asarray(inputs["edge_index"]).astype(np.int64)
    edge_attr = np.ascontiguousarray(np.asarray(inputs["edge_attr"], dtype=np.float32))

    T_w, EPW, src1, src23, drel, eaT = _host_prep(x, edge_index, edge_attr)

    f32 = lambda a: np.ascontiguousarray(np.asarray(a, dtype=np.float32))
    Wl1, bl1, Wr1, br1 = map(f32, (inputs["Wl1"], inputs["bl1"], inputs["Wr1"], inputs["br1"]))
    We1, att1, bo1 = map(f32, (inputs["We1"], inputs["att1"], inputs["bo1"]))
    Wl2, bl2, Wr2, br2 = map(f32, (inputs["Wl2"], inputs["bl2"], inputs["Wr2"], inputs["br2"]))
    We2, att2, bo2 = map(f32, (inputs["We2"], inputs["att2"], inputs["bo2"]))
    Wl3, bl3, Wr3, br3 = map(f32, (inputs["Wl3"], inputs["bl3"], inputs["Wr3"], inputs["br3"]))
    We3, att3, bo3 = map(f32, (inputs["We3"], inputs["att3"], inputs["bo3"]))
    Wres, bres = map(f32, (inputs["Wres"], inputs["bres"]))
    Wc1, bc1, Wc2, bc2 = map(f32, (inputs["Wc1"], inputs["bc1"], inputs["Wc2"], inputs["bc2"]))

    if T_w not in _cache:
        _cache[T_w] = _build_program(T_w)
    nc = _cache[T_w]

    common = {
        "x_full": x,
        "iotar": np.tile(np.arange(128, dtype=np.float32).reshape(1, 128), (128, 1)),
        "Wl1": Wl1, "Wr1": Wr1, "Wres": Wres, "We1": We1,
        "att1b": np.tile(att1.reshape(1, 1024), (128, 1)).astype(np.float32),
        "Wl2c": _chunks_for_rhs(Wl2), "Wr2c": _chunks_for_rhs(Wr2),
        "We2": We2, "att2b": np.tile(att2.reshape(1, 512), (128, 1)).astype(np.float32),
        "Wl3": Wl3, "Wr3": Wr3, "We3": We3,
        "att3b": np.tile(att3.reshape(1, 128), (128, 1)).astype(np.float32),
        "Wc1": Wc1, "Wc2": Wc2,
        "biasr1": (br1 + bl1).reshape(1, 1024),
        "const1": (bl1 + bo1 + bres).reshape(1, 1024),
        "biasr2": (br2 + bl2).reshape(1, 512),
        "const2b": np.tile((bl2.reshape(4, 128).mean(0) + bo2).reshape(1, 128),
                           (128, 1)).astype(np.float32),
        "biasr3": (br3 + bl3).reshape(1, 128),
        "const3b": np.tile((bl3 + bo3).reshape(1, 128), (128, 1)).astype(np.float32),
        "bc1": bc1.reshape(1, 64), "bc2": bc2.reshape(1, 3),
    }

    def tilemajor(a):
        return np.ascontiguousarray(
            a.reshape(W, T_w, 128).transpose(0, 2, 1).reshape(W * 128, T_w))

    in_maps = []
    for c in range(NC_CORES):
        m = dict(common)
        m["x_ownT"] = np.ascontiguousarray(_pad_own(x, c).T)
        m["src1i"] = np.ascontiguousarray(
            src1[c].astype(np.int32).reshape(W, T_w, 128).transpose(0, 2, 1)
            .reshape(W * 128, T_w))
        m["src23i"] = np.ascontiguousarray(
            src23[c].astype(np.int32).reshape(W, T_w, 128).transpose(0, 2, 1)
            .reshape(W * 128, T_w))
        m["drel"] = tilemajor(drel[c])
        m["eaT"] = np.ascontiguousarray(eaT[c].reshape(W * ED, EPW))
        in_maps.append(m)

    kernel._last_in_maps = in_maps
    res = run_bass_kernel_spmd(nc, in_maps, core_ids=list(range(NC_CORES)), trace=False)
    out = np.concatenate([res.results[c]["out_o"][:PER] for c in range(NC_CORES)], axis=0)
    if _DEBUG:
        kernel._last_results = res.results
    return out.astype(np.float32)



# revision 20
# speedup vs baseline: 4.8015x; 4.8015x over previous
"""Trainium2 Bass kernel for nn_EnhancedGAT (3-layer GATv2, N=10000, E=160000).

v2 strategy (8 NeuronCores, SPMD, dst-partitioned):
  - Balanced dst-bucketing: nodes are greedily assigned (LPT on in-degree) to
    80 buckets (8 cores x 10 windows) of exactly 128 nodes, equalizing edge
    counts -> T_w = ceil(max_bucket/128) = 17 instead of 18.
  - All matmuls in bf16 (4x PE throughput vs fp32); PSUM accumulates fp32.
  - Edge gathers via the CounterMachine dma_gather: ONE instruction per
    window (994ns fixed + 0.34ns/row) instead of one SWDGE op per 128-row
    tile. Layer-1 x rows are gathered twice: once plain (scatter rhs) and
    once transpose=True (matmul lhsT) -- kills the per-tile PE transpose.
  - Fused single pass per window: scores and weighted scatter happen in one
    loop over edge tiles (one gather, one one-hot generation per tile).
  - One-hot transposed (ohT) generated directly on DVE from a partition-
    broadcast drel row vs iota column (no PE transpose, no PSUM).
  - Segment softmax without segment-max: w = exp(logits); out = num/(den+eps).
  - Layer 1 G-trick: aggregate w-weighted one-hot @ raw x (128-dim), multiply
    by Wl1 once per window.
  - Layers 2/3 gather from bf16 xl tables built locally and AllGather'd
    (bf16 halves collective bytes); the gathered row is added into the score
    PSUM with an identity-matmul (PE) instead of a DVE add.
  - Elementwise work spread across DVE / Act / Pool to balance engines.
"""
import os
import numpy as np
import ml_dtypes

import concourse.bass as bass
import concourse.bacc as bacc
import concourse.mybir as mybir
import concourse.tile as tile
from concourse.bass_utils import run_bass_kernel_spmd
from concourse.masks import make_identity

F32 = mybir.dt.float32
F32R = mybir.dt.float32r
BF16 = mybir.dt.bfloat16
I16 = mybir.dt.int16
AF = mybir.ActivationFunctionType
OP = mybir.AluOpType
BF = ml_dtypes.bfloat16

NC_CORES = 8
N = 10000
ND = 128
ED = 32
PER = N // NC_CORES          # 1250 real nodes per core
NPAD = 1280
W = NPAD // 128              # 10 windows
EPS = 1e-16

_DEBUG = bool(int(os.environ.get("GAT_DEBUG", "0")))
_WLIM = int(os.environ.get("GAT_WLIM", str(W)))
_LAYERS = int(os.environ.get("GAT_LAYERS", "3"))
_NOCOLL = bool(int(os.environ.get("GAT_NOCOLL", "0")))
_REPEAT = int(os.environ.get("GAT_REPEAT", "1"))


# ----------------------------------------------------------------------------
# host-side prep
# ----------------------------------------------------------------------------

def _balanced_buckets(deg):
    """LPT: assign each node to one of 80 buckets (max 128 nodes each),
    minimizing max bucket edge count. Returns node2cws [N] -> (c, w, s)."""
    import heapq
    nb = NC_CORES * W
    order = np.argsort(-deg, kind="stable")
    heap = [(0, b) for b in range(nb)]
    heapq.heapify(heap)
    counts = np.zeros(nb, np.int64)
    slot_of = np.zeros(N, np.int64)
    bucket_of = np.zeros(N, np.int64)
    for n in order:
        while True:
            load, b = heapq.heappop(heap)
            if counts[b] < 128:
                break
        bucket_of[n] = b
        slot_of[n] = counts[b]
        counts[b] += 1
        heapq.heappush(heap, (load + int(deg[n]), b))
    return bucket_of, slot_of


def _idx16_wrap(idx):
    """[EPW] int -> [128, EPW//16] int16 in dma_gather layout (16-row wrap
    replicated across the 8 GpSimd banks)."""
    EPW = idx.shape[0]
    wrap = idx.reshape(EPW // 16, 16).T          # [16, EPW/16]
    return np.ascontiguousarray(np.tile(wrap, (8, 1)).astype(np.int16))


def _host_prep(x, edge_index, edge_attr):
    src = np.concatenate([edge_index[0], np.arange(N)]).astype(np.int64)
    dst = np.concatenate([edge_index[1], np.arange(N)]).astype(np.int64)
    ea = np.concatenate(
        [edge_attr, np.tile(edge_attr.mean(0), (N, 1))], axis=0
    ).astype(np.float32)

    deg = np.bincount(dst, minlength=N)
    bucket_of, slot_of = _balanced_buckets(deg)

    key = bucket_of[dst]
    order = np.argsort(key, kind="stable")
    counts = np.bincount(key[order], minlength=NC_CORES * W)
    T_w = int(np.ceil(counts.max() / 128))
    EPW = T_w * 128

    starts = np.zeros(NC_CORES * W, np.int64)
    starts[1:] = np.cumsum(counts)[:-1]

    # src -> row in the AllGather'd xl tables
    c_of = bucket_of // W
    w_of = bucket_of % W
    row23 = c_of * NPAD + w_of * 128 + slot_of          # [N]

    src1 = np.zeros((NC_CORES, W, EPW), np.int64)
    src23 = np.zeros((NC_CORES, W, EPW), np.int64)
    drel = np.full((NC_CORES, W, EPW), -1.0, np.float32)
    eaT = np.zeros((NC_CORES, W, ED, EPW), np.float32)
    for b in range(NC_CORES * W):
        c, w = b // W, b % W
        k = int(counts[b])
        m = order[starts[b]: starts[b] + k]
        src1[c, w, :k] = src[m]
        src23[c, w, :k] = row23[src[m]]
        drel[c, w, :k] = slot_of[dst[m]].astype(np.float32)
        eaT[c, w, :, :k] = ea[m].T

    # node order per core: nodes[c, w*128+s]
    node_of = np.full((NC_CORES, NPAD), 0, np.int64)
    nn = np.arange(N)
    node_of[c_of[nn], w_of[nn] * 128 + slot_of[nn]] = nn
    valid = np.zeros((NC_CORES, NPAD), bool)
    valid[c_of[nn], w_of[nn] * 128 + slot_of[nn]] = True

    return T_w, EPW, src1, src23, drel, eaT, node_of, valid


def _chunks_for_rhs(Wm):
    """[K, F] weight -> [128, (K//128)*F]: chunk k at cols [k*F:(k+1)*F]."""
    K, F = Wm.shape
    assert K % 128 == 0
    return np.ascontiguousarray(
        Wm.reshape(K // 128, 128, F).transpose(1, 0, 2).reshape(128, -1)
    )


def _bf(a):
    return np.ascontiguousarray(np.asarray(a, np.float32)).astype(BF)


# ----------------------------------------------------------------------------
# bass program
# ----------------------------------------------------------------------------

def _build_program(T_w, use_bias=False, repeat=None):
    repeat = _REPEAT if repeat is None else repeat
    EPW = T_w * 128
    EPW16 = EPW // 16
    nc = bacc.Bacc("TRN2", target_bir_lowering=False, debug=False,
                   enable_asserts=False, num_devices=NC_CORES)

    def din(name, shape, dt=BF16):
        return nc.dram_tensor(name, shape, dt, kind="ExternalInput")

    x_full = din("x_full", [N, ND])
    x_f32_d = din("x_f32", [N, ND], F32)
    x_ownT_d = din("x_ownT", [ND, NPAD], F32R)
    src1_d = din("src1i", [W * 128, EPW16], I16)
    src23_d = din("src23i", [W * 128, EPW16], I16)
    drel_d = din("drel", [W * 128, T_w], F32)
    drelR_d = din("drelR", [W, EPW])
    eaT_d = din("eaT", [W * ED, EPW])
    iotar_d = din("iotar", [128, 128], F32)
    iotap_d = din("iotap", [128, 1], F32)

    Wl1_d = din("Wl1", [128, 1024])
    Wl1r_d = din("Wl1r", [128, 1024], F32R)
    Wr1_d = din("Wr1", [128, 1024], F32R)
    Wres_d = din("Wres", [128, 1024], F32R)
    We1_d = din("We1", [ED, 1024])
    att1_d = din("att1b", [128, 1024])
    Wl2_d = din("Wl2c", [128, 8 * 512], F32R)
    Wr2_d = din("Wr2c", [128, 8 * 512], F32R)
    We2_d = din("We2", [ED, 512])
    att2_d = din("att2b", [128, 512])
    Wl3_d = din("Wl3", [128, 128], F32R)
    Wr3_d = din("Wr3", [128, 128], F32R)
    We3_d = din("We3", [ED, 128])
    att3_d = din("att3b", [128, 128])
    Wc1_d = din("Wc1", [128, 64], F32R)
    Wc2_d = din("Wc2", [64, 4], F32R)
    if use_bias:
        biasr1_d = din("biasr1", [1, 1024], F32R)
        const1_d = din("const1", [1, 1024], F32R)
        biasr2_d = din("biasr2", [1, 512], F32R)
        const2_d = din("const2b", [128, 128], F32)
        biasr3_d = din("biasr3", [1, 128], F32R)
        const3_d = din("const3b", [128, 128], F32)
        bc1_d = din("bc1", [1, 64], F32R)
        bc2_d = din("bc2", [1, 3], F32R)

    out_d = nc.dram_tensor("out_o", [NPAD, 3], F32, kind="ExternalOutput")
    if _DEBUG:
        h1_dbg = nc.dram_tensor("h1_dbg", [NPAD, 1024], F32, kind="ExternalOutput")
        h2_dbg = nc.dram_tensor("h2_dbg", [NPAD, 128], F32, kind="ExternalOutput")
        h3_dbg = nc.dram_tensor("h3_dbg", [NPAD, 128], F32, kind="ExternalOutput")

    with tile.TileContext(nc) as tc:
        with tc.tile_pool(name="wp", bufs=1) as wp, \
             tc.tile_pool(name="slab", bufs=1) as slab, \
             tc.tile_pool(name="io", bufs=2) as io, \
             tc.tile_pool(name="io3", bufs=3) as io3, \
             tc.tile_pool(name="fat", bufs=2) as fat, \
             tc.tile_pool(name="tp1", bufs=1) as tp1, \
             tc.tile_pool(name="big", bufs=2) as big, \
             tc.tile_pool(name="big1", bufs=1) as big1, \
             tc.tile_pool(name="psS", bufs=2, space="PSUM") as psS, \
             tc.tile_pool(name="psG", bufs=1, space="PSUM") as psG, \
             tc.tile_pool(name="psT", bufs=1, space="PSUM") as psT, \
             tc.tile_pool(name="dram", bufs=1, space="DRAM") as dr:

            # ---------- resident constants ----------
            def load(dram_t, shape, name, dt=BF16):
                t = wp.tile(shape, dt, name=name, tag=name)
                nc.sync.dma_start(out=t[:], in_=dram_t.ap())
                return t

            x_ownT = load(x_ownT_d, [ND, NPAD], "x_ownT", dt=F32R)
            Wl1 = load(Wl1_d, [128, 1024], "Wl1")
            Wl1r = load(Wl1r_d, [128, 1024], "Wl1r", dt=F32R)
            Wr1 = load(Wr1_d, [128, 1024], "Wr1", dt=F32R)
            Wres = load(Wres_d, [128, 1024], "Wres", dt=F32R)
            We1 = load(We1_d, [ED, 1024], "We1")
            att1b = load(att1_d, [128, 1024], "att1b")
            Wl2 = load(Wl2_d, [128, 8 * 512], "Wl2", dt=F32R)
            Wr2 = load(Wr2_d, [128, 8 * 512], "Wr2", dt=F32R)
            We2 = load(We2_d, [ED, 512], "We2")
            att2b = load(att2_d, [128, 512], "att2b")
            Wl3 = load(Wl3_d, [128, 128], "Wl3", dt=F32R)
            Wr3 = load(Wr3_d, [128, 128], "Wr3", dt=F32R)
            We3 = load(We3_d, [ED, 128], "We3")
            att3b = load(att3_d, [128, 128], "att3b")
            Wc1 = load(Wc1_d, [128, 64], "Wc1", dt=F32R)
            Wc2 = load(Wc2_d, [64, 4], "Wc2", dt=F32R)
            iotar = load(iotar_d, [128, 128], "iotar", dt=F32)
            iotap = load(iotap_d, [128, 1], "iotap", dt=F32)
            if use_bias:
                biasr1 = load(biasr1_d, [1, 1024], "biasr1", dt=F32R)
                const1 = load(const1_d, [1, 1024], "const1", dt=F32R)
                biasr2 = load(biasr2_d, [1, 512], "biasr2", dt=F32R)
                const2b = load(const2_d, [128, 128], "const2b", dt=F32)
                biasr3 = load(biasr3_d, [1, 128], "biasr3", dt=F32R)
                const3b = load(const3_d, [128, 128], "const3b", dt=F32)
                bc1 = load(bc1_d, [1, 64], "bc1", dt=F32R)
                bc2 = load(bc2_d, [1, 3], "bc2", dt=F32R)
                ones1 = wp.tile([1, 128], F32R, name="ones1", tag="ones1")
                nc.vector.memset(ones1[:], 1.0)

            identf = wp.tile([128, 128], F32, name="identf", tag="identf")
            make_identity(nc, identf[:])
            ident = wp.tile([128, 128], F32R, name="ident", tag="ident")
            nc.vector.tensor_copy(out=ident[:], in_=identf[:])

            xr2_own = slab.tile([128, W * 512], BF16, name="xr2_own", tag="xr2_own")
            xr3_own = slab.tile([128, W * 128], BF16, name="xr3_own", tag="xr3_own")

            # ---------- helpers ----------
            def window_meta(w, lidx, layer):
                """Load per-window metadata; returns (idx, drel_w, drelB, ea_w)."""
                src_d = src1_d if layer == 1 else src23_d
                idx_w = io.tile([128, EPW16], I16, name=f"idx{lidx}_{w}", tag="idx")
                nc.sync.dma_start(out=idx_w[:], in_=src_d.ap()[w * 128:(w + 1) * 128, :])
                drel_w = io.tile([128, T_w], F32, name=f"drel{lidx}_{w}", tag="drel")
                nc.sync.dma_start(out=drel_w[:], in_=drel_d.ap()[w * 128:(w + 1) * 128, :])
                drelr = io.tile([1, EPW], BF16, name=f"drelr{lidx}_{w}", tag="drelr")
                nc.sync.dma_start(out=drelr[:], in_=drelR_d.ap()[w:w + 1, :])
                drelB = big.tile([128, EPW], BF16, name=f"drelB{lidx}_{w}", tag="drelB", bufs=1)
                nc.gpsimd.partition_broadcast(drelB[:], drelr[:])
                ea_w = big.tile([ED, EPW], BF16, name=f"ea{lidx}_{w}", tag="ea")
                nc.sync.dma_start(out=ea_w[:], in_=eaT_d.ap()[w * ED:(w + 1) * ED, :])
                return idx_w, drel_w, drelB, ea_w

            def gather(idx_w, table_ap, Fdim, pool, tag, name, transpose=False,
                       dt=BF16):
                """dma_gather for the window in chunks of <=6 tiles (the SWDGE
                per-engine ring holds 64 descriptors; rows/16+2 must stay
                under it, so <=896 rows per call)."""
                CH = 6
                if transpose:
                    g = pool.tile([128, (Fdim // 128) * EPW], dt, name=name, tag=tag)
                else:
                    g = pool.tile([128, T_w * Fdim], dt, name=name, tag=tag)
                t0 = 0
                while t0 < T_w:
                    tl = min(CH, T_w - t0)
                    L = tl * 128
                    idx_sl = idx_w[:, t0 * 8:(t0 + tl) * 8]
                    if transpose:
                        ap3 = g[:, t0 * 128:(t0 + tl) * 128].rearrange(
                            "p (c e) -> p c e", c=Fdim // 128)
                    else:
                        ap3 = g[:, t0 * Fdim:(t0 + tl) * Fdim].rearrange(
                            "p (t e) -> p t e", t=tl)
                    nc.gpsimd.dma_gather(ap3, table_ap, idx_sl, L, L, Fdim,
                                         transpose=transpose)
                    t0 += tl
                return g

            def onehot_pair(drel_w, drelB, t, w, lidx):
                """oh[e,d] on DVE (iota row vs drel scalar) and ohT[d,e] on DVE
                (drel bcast row vs iota col). Both bf16."""
                oh = io3.tile([128, 128], F32R, name=f"oh{lidx}_{w}_{t}", tag="oh")
                nc.vector.tensor_scalar(out=oh[:], in0=iotar[:],
                                        scalar1=drel_w[:, t:t + 1], scalar2=None,
                                        op0=OP.is_equal)
                ohT = io3.tile([128, 128], BF16, name=f"ohT{lidx}_{w}_{t}", tag="ohT")
                nc.vector.tensor_scalar(out=ohT[:], in0=drelB[:, t * 128:(t + 1) * 128],
                                        scalar1=iotap[:, 0:1], scalar2=None,
                                        op0=OP.is_equal)
                return oh, ohT

            def rz_from(ps_z, H, w, lidx, quarter=False):
                zt = io.tile([128, H], F32, name=f"zt{lidx}_{w}", tag="zt")
                nc.vector.tensor_scalar(out=zt[:], in0=ps_z, scalar1=EPS,
                                        scalar2=None, op0=OP.add)
                rz = io.tile([128, H], F32, name=f"rz{lidx}_{w}", tag="rz")
                nc.vector.reciprocal(out=rz[:], in_=zt[:])
                if quarter:
                    nc.vector.tensor_scalar(out=rz[:], in0=rz[:], scalar1=0.25,
                                            scalar2=None, op0=OP.mult)
                return rz

            def elu_of(a, F, w, lidx, out_dt=F32R):
                """h = elu(a) = relu(a) + min(exp(a)-1, 0); f32 value path."""
                ex = io.tile([128, F], F32, name=f"ex{lidx}_{w}", tag="ex")
                nc.scalar.activation(out=ex[:], in_=a[:], func=AF.Exp)
                em = io.tile([128, F], F32, name=f"em{lidx}_{w}", tag="em")
                nc.vector.tensor_scalar(out=em[:], in0=ex[:], scalar1=1.0, scalar2=0.0,
                                        op0=OP.subtract, op1=OP.min)
                r = io.tile([128, F], F32, name=f"r{lidx}_{w}", tag="r")
                nc.scalar.activation(out=r[:], in_=a[:], func=AF.Relu)
                h = io.tile([128, F], out_dt, name=f"h{lidx}_{w}", tag="helu")
                nc.vector.tensor_tensor(out=h[:], in0=r[:], in1=em[:], op=OP.add)
                return h

            def transpose_to(sb_in, F, w, lidx, tag):
                """f32r [128, F] -> transposed f32r [128, F] via PE, copies
                alternating Act/DVE."""
                tout = tp1.tile([128, F], F32R, name=f"T{tag}{lidx}_{w}", tag="Ttmp")
                for k in range(F // 128):
                    ps = psT.tile([128, 128], F32R, name=f"ps{tag}{lidx}_{w}_{k}",
                                  tag="scrr")
                    nc.tensor.transpose(out=ps[:], in_=sb_in[:, k * 128:(k + 1) * 128],
                                        identity=ident[:])
                    osl = tout[:, k * 128:(k + 1) * 128]
                    if k % 2 == 0:
                        nc.scalar.copy(out=osl, in_=ps[:])
                    else:
                        nc.vector.tensor_copy(out=osl, in_=ps[:])
                return tout

            for rep in range(repeat):
                xl2_bounce = dr.tile([NPAD, 512], F32, name=f"xl2_bounce_{rep}",
                                     tag=f"x2b{rep}")
                xl2_full = dr.tile([NC_CORES * NPAD, 512], F32,
                                   name=f"xl2_full_{rep}", addr_space="Shared",
                                   tag=f"x2f{rep}")
                xl3_bounce = dr.tile([NPAD, 128], F32, name=f"xl3_bounce_{rep}",
                                     tag=f"x3b{rep}")
                xl3_full = dr.tile([NC_CORES * NPAD, 128], F32,
                                   name=f"xl3_full_{rep}", addr_space="Shared",
                                   tag=f"x3f{rep}")

                # =========================================================
                # LAYER 1 (H=8, G-trick) + xl2/xr2 builds, fused single pass
                # =========================================================
                for w in range(_WLIM):
                    idx_w, drel_w, drelB, ea_w = window_meta(w, 1, 1)
                    xgT = gather(idx_w, x_full.ap(), 128, big, "gxT", f"gxT1_{w}",
                                 transpose=True)
                    xga = gather(idx_w, x_f32_d.ap(), 128, big, "gxa", f"gxa1_{w}",
                                 dt=F32)

                    # xr1 window table: x_own_w @ Wr1 (+bias)
                    ps_xr = psS.tile([128, 1024], F32, name=f"psxr1_{w}", tag="S")
                    for j in range(2):
                        sl = slice(j * 512, (j + 1) * 512)
                        nc.tensor.matmul(out=ps_xr[:, sl],
                                         lhsT=x_ownT[:, w * 128:(w + 1) * 128],
                                         rhs=Wr1[:, sl], start=True, stop=not use_bias)
                        if use_bias:
                            nc.tensor.matmul(out=ps_xr[:, sl], lhsT=ones1[:],
                                             rhs=biasr1[:, sl], start=False, stop=True)
                    xr1sb = fat.tile([128, 1024], BF16, name=f"xr1_{w}", tag="xr1")
                    nc.scalar.copy(out=xr1sb[:], in_=ps_xr[:])

                    we_w = io.tile([128, T_w * 8], F32, name=f"wew1_{w}", tag="wew")
                    ps_G = psG.tile([128, 1024], F32, name=f"psG_{w}", tag="G")
                    scrW = psT.tile([128, 8], F32, name=f"scrW1_{w}", tag="scrW")

                    for t in range(T_w):
                        oh, ohT = onehot_pair(drel_w, drelB, t, w, 1)
                        ps_S = psS.tile([128, 1024], F32, name=f"psS1_{w}_{t}", tag="S")
                        for j in range(2):
                            sl = slice(j * 512, (j + 1) * 512)
                            nc.tensor.matmul(out=ps_S[:, sl],
                                             lhsT=xgT[:, t * 128:(t + 1) * 128],
                                             rhs=Wl1[:, sl], start=True, stop=False)
                            nc.tensor.matmul(out=ps_S[:, sl],
                                             lhsT=ea_w[:, t * 128:(t + 1) * 128],
                                             rhs=We1[:, sl], start=False, stop=False)
                            nc.tensor.matmul(out=ps_S[:, sl], lhsT=ohT[:],
                                             rhs=xr1sb[:, sl], start=False, stop=True)
                        # s = att * lrelu(ps_S)
                        s = fat.tile([128, 1024], BF16, name=f"s1_{w}_{t}", tag="lk1024")
                        nc.scalar.activation(out=s[:], in_=ps_S[:], func=AF.Prelu,
                                             alpha=0.2)
                        nc.vector.tensor_tensor(out=s[:], in0=s[:], in1=att1b[:],
                                                op=OP.mult)
                        # per-head reduce (bf16 out: fast DVE mode; logits are
                        # O(1) so bf16 accumulation noise is ~0.5%)
                        lg = io.tile([128, 8], BF16, name=f"lg1_{w}_{t}", tag="lg")
                        uv = s[:].rearrange("p (h c) -> p h c", h=8)
                        with nc.allow_low_precision(reason="bf16 logits"):
                            nc.vector.tensor_reduce(out=lg[:], in_=uv,
                                                    axis=mybir.AxisListType.X,
                                                    op=OP.add)
                        nc.scalar.activation(out=we_w[:, t * 8:(t + 1) * 8], in_=lg[:],
                                             func=AF.Exp)
                        # scatter rhs: re[:, h*128:(h+1)*128] = w_h * xga_t
                        re = fat.tile([128, 1032], F32R, name=f"re_{w}_{t}", tag="vr1024")
                        for h in range(8):
                            xsl = xga[:, t * 128:(t + 1) * 128]
                            osl = re[:, h * 128:(h + 1) * 128]
                            wsl = we_w[:, t * 8 + h:t * 8 + h + 1]
                            if h % 2 == 0:
                                nc.scalar.activation(out=osl, in_=xsl, func=AF.Copy,
                                                     scale=wsl)
                            else:
                                nc.vector.tensor_scalar(out=osl, in0=xsl, scalar1=wsl,
                                                        scalar2=None, op0=OP.mult)
                        nc.gpsimd.tensor_copy(out=re[:, 1024:1032],
                                              in_=we_w[:, t * 8:(t + 1) * 8])
                        for j in range(2):
                            sl = slice(j * 512, (j + 1) * 512)
                            nc.tensor.matmul(out=ps_G[:, sl], lhsT=oh[:],
                                             rhs=re[:, sl],
                                             start=(t == 0), stop=(t == T_w - 1))
                        nc.tensor.matmul(out=scrW[:], lhsT=oh[:], rhs=re[:, 1024:1032],
                                         start=(t == 0), stop=(t == T_w - 1))

                    # ---- window flush ----
                    rz = rz_from(scrW[:], 8, w, 1)
                    gn = fat.tile([128, 1024], F32R, name=f"gn_{w}", tag="vr1024")
                    for h in range(8):
                        nc.scalar.activation(out=gn[:, h * 128:(h + 1) * 128],
                                             in_=ps_G[:, h * 128:(h + 1) * 128],
                                             func=AF.Copy, scale=rz[:, h:h + 1])
                    gnT = transpose_to(gn, 1024, w, 1, "g")
                    ps_O = psS.tile([128, 1024], F32, name=f"psO1_{w}", tag="S")
                    for h in range(8):
                        nc.tensor.matmul(out=ps_O[:, h * 128:(h + 1) * 128],
                                         lhsT=gnT[:, h * 128:(h + 1) * 128],
                                         rhs=Wl1r[:, h * 128:(h + 1) * 128],
                                         start=(h % 4 == 0), stop=False)
                    for j in range(2):
                        sl = slice(j * 512, (j + 1) * 512)
                        nc.tensor.matmul(out=ps_O[:, sl],
                                         lhsT=x_ownT[:, w * 128:(w + 1) * 128],
                                         rhs=Wres[:, sl], start=False,
                                         stop=not use_bias)
                        if use_bias:
                            nc.tensor.matmul(out=ps_O[:, sl], lhsT=ones1[:],
                                             rhs=const1[:, sl], start=False, stop=True)
                    h1_w = fat.tile([128, 1024], F32R, name=f"h1_{w}", tag="vr1024")
                    nc.scalar.activation(out=h1_w[:], in_=ps_O[:], func=AF.Prelu,
                                         alpha=0.2)
                    if _DEBUG:
                        nc.sync.dma_start(out=h1_dbg.ap()[w * 128:(w + 1) * 128, :],
                                          in_=h1_w[:].bitcast(F32))
                    h1T = transpose_to(h1_w, 1024, w, 1, "h")
                    ps_x2f = psS.tile([128, 1024], F32, name=f"psx2_{w}", tag="S")
                    ps_x2 = ps_x2f[:, 0:512]
                    for k in range(8):
                        nc.tensor.matmul(out=ps_x2, lhsT=h1T[:, k * 128:(k + 1) * 128],
                                         rhs=Wl2[:, k * 512:(k + 1) * 512],
                                         start=(k == 0), stop=(k == 7))
                    xl2_w = io.tile([128, 512], F32, name=f"xl2_{w}", tag="xl2")
                    nc.scalar.copy(out=xl2_w[:], in_=ps_x2)
                    nc.sync.dma_start(out=xl2_bounce[w * 128:(w + 1) * 128, :],
                                      in_=xl2_w[:])
                    ps_r2f = psS.tile([128, 1024], F32, name=f"psr2_{w}", tag="S")
                    ps_r2 = ps_r2f[:, 0:512]
                    for k in range(8):
                        nc.tensor.matmul(out=ps_r2, lhsT=h1T[:, k * 128:(k + 1) * 128],
                                         rhs=Wr2[:, k * 512:(k + 1) * 512],
                                         start=(k == 0), stop=(k == 7) and not use_bias)
                    if use_bias:
                        nc.tensor.matmul(out=ps_r2, lhsT=ones1[:], rhs=biasr2[:],
                                         start=False, stop=True)
                    nc.scalar.copy(out=xr2_own[:, w * 512:(w + 1) * 512], in_=ps_r2)

                # =========================================================
                # LAYER 2 (H=4, gather xl2), fused single pass
                # =========================================================
                if _LAYERS >= 2:
                    if _NOCOLL:
                        nc.sync.dma_start(out=xl2_full[0:NPAD, :], in_=xl2_bounce[:])
                    else:
                        nc.gpsimd.collective_compute(
                            "AllGather", OP.bypass,
                            replica_groups=[list(range(NC_CORES))],
                            ins=[xl2_bounce[:]], outs=[xl2_full[:]])

                    for w in range(_WLIM):
                        idx_w, drel_w, drelB, ea_w = window_meta(w, 2, 2)
                        g2 = gather(idx_w, xl2_full[:], 512, big1, "g2", f"g2_{w}", dt=F32)
                        we_w = io.tile([128, T_w * 4], F32, name=f"wew2_{w}", tag="wew")
                        ps_O2 = psG.tile([128, 1024], F32, name=f"psO2_{w}", tag="G")
                        scrW = psT.tile([128, 8], F32, name=f"scrW2_{w}", tag="scrW")

                        for t in range(T_w):
                            oh, ohT = onehot_pair(drel_w, drelB, t, w, 2)
                            ps_Bf = psS.tile([128, 1024], F32, name=f"psB2_{w}_{t}",
                                             tag="S")
                            ps_B = ps_Bf[:, 0:512]
                            nc.tensor.matmul(out=ps_B,
                                             lhsT=ea_w[:, t * 128:(t + 1) * 128],
                                             rhs=We2[:], start=True, stop=False)
                            nc.tensor.matmul(out=ps_B, lhsT=ohT[:],
                                             rhs=xr2_own[:, w * 512:(w + 1) * 512],
                                             start=False, stop=True)
                            spre = fat.tile([128, 512], BF16, name=f"spre2_{w}_{t}",
                                            tag="sp512")
                            nc.vector.tensor_tensor(out=spre[:], in0=ps_B,
                                                    in1=g2[:, t * 512:(t + 1) * 512],
                                                    op=OP.add)
                            s = fat.tile([128, 512], BF16, name=f"s2_{w}_{t}",
                                         tag="lk512")
                            nc.scalar.activation(out=s[:], in_=spre[:], func=AF.Prelu,
                                                 alpha=0.2)
                            nc.vector.tensor_tensor(out=s[:], in0=s[:], in1=att2b[:],
                                                    op=OP.mult)
                            lg = io.tile([128, 4], BF16, name=f"lg2_{w}_{t}", tag="lg")
                            uv = s[:].rearrange("p (h c) -> p h c", h=4)
                            with nc.allow_low_precision(reason="bf16 logits"):
                                nc.vector.tensor_reduce(out=lg[:], in_=uv,
                                                        axis=mybir.AxisListType.X,
                                                        op=OP.add)
                            nc.scalar.activation(out=we_w[:, t * 4:(t + 1) * 4],
                                                 in_=lg[:], func=AF.Exp)
                            At = fat.tile([128, 516], F32R, name=f"At2_{w}_{t}",
                                          tag="At2")
                            for h in range(4):
                                src_sl = g2[:, t * 512 + h * 128: t * 512 + (h + 1) * 128]
                                osl = At[:, h * 128:(h + 1) * 128]
                                wsl = we_w[:, t * 4 + h:t * 4 + h + 1]
                                if h % 2 == 0:
                                    nc.scalar.activation(out=osl, in_=src_sl,
                                                         func=AF.Copy, scale=wsl)
                                else:
                                    nc.vector.tensor_scalar(out=osl, in0=src_sl,
                                                            scalar1=wsl, scalar2=None,
                                                            op0=OP.mult)
                            nc.gpsimd.tensor_copy(out=At[:, 512:516],
                                                  in_=we_w[:, t * 4:(t + 1) * 4])
                            nc.tensor.matmul(out=ps_O2[:, 0:512], lhsT=oh[:], rhs=At[:, 0:512],
                                             start=(t == 0), stop=(t == T_w - 1))
                            nc.tensor.matmul(out=scrW[:, 0:4], lhsT=oh[:],
                                             rhs=At[:, 512:516],
                                             start=(t == 0), stop=(t == T_w - 1))

                        # ---- flush: mean over heads + elu + xl3/xr3 builds ----
                        rz = rz_from(scrW[:, 0:4], 4, w, 2, quarter=True)
                        m4 = io.tile([128, 512], F32, name=f"m4_{w}", tag="xl2")
                        for h in range(4):
                            nc.scalar.activation(out=m4[:, h * 128:(h + 1) * 128],
                                                 in_=ps_O2[:, h * 128:(h + 1) * 128],
                                                 func=AF.Copy, scale=rz[:, h:h + 1])
                        m01 = io.tile([128, 128], F32, name=f"m01_{w}", tag="m01")
                        nc.vector.tensor_tensor(out=m01[:], in0=m4[:, 0:128],
                                                in1=m4[:, 128:256], op=OP.add)
                        m23 = io.tile([128, 128], F32, name=f"m23_{w}", tag="m23")
                        nc.vector.tensor_tensor(out=m23[:], in0=m4[:, 256:384],
                                                in1=m4[:, 384:512], op=OP.add)
                        a2 = io.tile([128, 128], F32, name=f"a2_{w}", tag="a2")
                        nc.vector.tensor_tensor(out=a2[:], in0=m01[:], in1=m23[:],
                                                op=OP.add)
                        if use_bias:
                            nc.vector.tensor_tensor(out=a2[:], in0=a2[:], in1=const2b[:],
                                                    op=OP.add)
                        h2_w = elu_of(a2, 128, w, 2)
                        if _DEBUG:
                            nc.sync.dma_start(out=h2_dbg.ap()[w * 128:(w + 1) * 128, :],
                                              in_=h2_w[:].bitcast(F32))
                        h2T = transpose_to(h2_w, 128, w, 2, "h2")
                        ps_x3f = psS.tile([128, 1024], F32, name=f"psx3_{w}", tag="S")
                        ps_x3 = ps_x3f[:, 0:128]
                        nc.tensor.matmul(out=ps_x3, lhsT=h2T[:], rhs=Wl3[:],
                                         start=True, stop=True)
                        xl3_w = io.tile([128, 128], F32, name=f"xl3_{w}", tag="xl3")
                        nc.scalar.copy(out=xl3_w[:], in_=ps_x3)
                        nc.sync.dma_start(out=xl3_bounce[w * 128:(w + 1) * 128, :],
                                          in_=xl3_w[:])
                        ps_r3f = psS.tile([128, 1024], F32, name=f"psr3_{w}", tag="S")
                        ps_r3 = ps_r3f[:, 0:128]
                        nc.tensor.matmul(out=ps_r3, lhsT=h2T[:], rhs=Wr3[:],
                                         start=True, stop=not use_bias)
                        if use_bias:
                            nc.tensor.matmul(out=ps_r3, lhsT=ones1[:], rhs=biasr3[:],
                                             start=False, stop=True)
                        nc.scalar.copy(out=xr3_own[:, w * 128:(w + 1) * 128],
                                       in_=ps_r3)

                # =========================================================
                # LAYER 3 (H=1) + head, fused single pass
                # =========================================================
                if _LAYERS >= 3:
                    if _NOCOLL:
                        nc.sync.dma_start(out=xl3_full[0:NPAD, :], in_=xl3_bounce[:])
                    else:
                        nc.gpsimd.collective_compute(
                            "AllGather", OP.bypass,
                            replica_groups=[list(range(NC_CORES))],
                            ins=[xl3_bounce[:]], outs=[xl3_full[:]])

                    for w in range(_WLIM):
                        idx_w, drel_w, drelB, ea_w = window_meta(w, 3, 3)
                        g3 = gather(idx_w, xl3_full[:], 128, big, "gxa", f"g3_{w}", dt=F32)
                        we_w = io.tile([128, T_w], F32, name=f"wew3_{w}", tag="wew")
                        ps_O3 = psG.tile([128, 1024], F32, name=f"psO3_{w}", tag="G")
                        scrW = psT.tile([128, 8], F32, name=f"scrW3_{w}", tag="scrW")

                        for t in range(T_w):
                            oh, ohT = onehot_pair(drel_w, drelB, t, w, 3)
                            ps_Bf = psS.tile([128, 1024], F32, name=f"psB3_{w}_{t}",
                                             tag="S")
                            ps_B = ps_Bf[:, 0:128]
                            nc.tensor.matmul(out=ps_B,
                                             lhsT=ea_w[:, t * 128:(t + 1) * 128],
                                             rhs=We3[:], start=True, stop=False)
                            nc.tensor.matmul(out=ps_B, lhsT=ohT[:],
                                             rhs=xr3_own[:, w * 128:(w + 1) * 128],
                                             start=False, stop=True)
                            spre = fat.tile([128, 128], BF16, name=f"spre3_{w}_{t}",
                                            tag="sp128")
                            nc.vector.tensor_tensor(out=spre[:], in0=ps_B,
                                                    in1=g3[:, t * 128:(t + 1) * 128],
                                                    op=OP.add)
                            s = fat.tile([128, 128], BF16, name=f"s3_{w}_{t}",
                                         tag="lk128")
                            nc.scalar.activation(out=s[:], in_=spre[:], func=AF.Prelu,
                                                 alpha=0.2)
                            nc.vector.tensor_tensor(out=s[:], in0=s[:], in1=att3b[:],
                                                    op=OP.mult)
                            lg = io.tile([128, 1], F32, name=f"lg3_{w}_{t}", tag="lg")
                            nc.vector.tensor_reduce(out=lg[:], in_=s[:],
                                                    axis=mybir.AxisListType.X,
                                                    op=OP.add)
                            nc.scalar.activation(out=we_w[:, t:t + 1], in_=lg[:],
                                                 func=AF.Exp)
                            At = io3.tile([128, 130], F32R, name=f"At3_{w}_{t}",
                                          tag="At3")
                            nc.scalar.activation(out=At[:, 0:128],
                                                 in_=g3[:, t * 128:(t + 1) * 128],
                                                 func=AF.Copy, scale=we_w[:, t:t + 1])
                            nc.gpsimd.tensor_copy(
                                out=At[:, 128:130],
                                in_=we_w[:, t:t + 1].broadcast_to((128, 2)))
                            nc.tensor.matmul(out=ps_O3[:, 0:128], lhsT=oh[:], rhs=At[:, 0:128],
                                             start=(t == 0), stop=(t == T_w - 1))
                            nc.tensor.matmul(out=scrW[:, 0:2], lhsT=oh[:],
                                             rhs=At[:, 128:130],
                                             start=(t == 0), stop=(t == T_w - 1))

                        # ---- flush + head ----
                        rz = rz_from(scrW[:, 0:1], 1, w, 3)
                        o3 = io.tile([128, 128], F32, name=f"o3_{w}", tag="o3")
                        nc.scalar.activation(out=o3[:], in_=ps_O3[:, 0:128],
                                             func=AF.Copy, scale=rz[:, 0:1])
                        if use_bias:
                            nc.vector.tensor_tensor(out=o3[:], in0=o3[:], in1=const3b[:],
                                                    op=OP.add)
                        h3_w = elu_of(o3, 128, w, 3)
                        if _DEBUG:
                            nc.sync.dma_start(out=h3_dbg.ap()[w * 128:(w + 1) * 128, :],
                                              in_=h3_w[:].bitcast(F32))
                        h3T = transpose_to(h3_w, 128, w, 3, "h3")
                        ps_c1f = psS.tile([128, 1024], F32, name=f"psc1_{w}", tag="S")
                        ps_c1 = ps_c1f[:, 0:64]
                        nc.tensor.matmul(out=ps_c1, lhsT=h3T[:], rhs=Wc1[:],
                                         start=True, stop=not use_bias)
                        if use_bias:
                            nc.tensor.matmul(out=ps_c1, lhsT=ones1[:], rhs=bc1[:],
                                             start=False, stop=True)
                        a1 = io.tile([128, 64], F32, name=f"a1_{w}", tag="a1")
                        nc.scalar.copy(out=a1[:], in_=ps_c1)
                        c1 = elu_of(a1, 64, w, 4)
                        ps_t = psT.tile([128, 128], F32R, name=f"psct_{w}", tag="scrr")
                        nc.tensor.transpose(out=ps_t[0:64, :], in_=c1[:],
                                            identity=ident[:])
                        c1T = io.tile([64, 128], F32R, name=f"c1T_{w}", tag="c1T")
                        nc.scalar.copy(out=c1T[:], in_=ps_t[0:64, :])
                        ps_ff = psS.tile([128, 1024], F32, name=f"psf_{w}", tag="S")
                        ps_f = ps_ff[:, 0:4]
                        nc.tensor.matmul(out=ps_f, lhsT=c1T[:], rhs=Wc2[:],
                                         start=True, stop=not use_bias)
                        if use_bias:
                            nc.tensor.matmul(out=ps_f, lhsT=ones1[:], rhs=bc2[:],
                                             start=False, stop=True)
                        fo = io.tile([128, 3], F32, name=f"fo_{w}", tag="fo")
                        nc.scalar.copy(out=fo[:], in_=ps_ff[:, 0:3])
                        nc.sync.dma_start(out=out_d.ap()[w * 128:(w + 1) * 128, :],
                                          in_=fo[:])

    nc.compile()
    return nc


# ----------------------------------------------------------------------------
# entry point
# ----------------------------------------------------------------------------

_cache = {}


def kernel(**inputs):
    x = np.ascontiguousarray(np.asarray(inputs["x"], dtype=np.float32))
    edge_index = np.asarray(inputs["edge_index"]).astype(np.int64)
    edge_attr = np.ascontiguousarray(np.asarray(inputs["edge_attr"], dtype=np.float32))

    T_w, EPW, src1, src23, drel, eaT, node_of, valid = _host_prep(
        x, edge_index, edge_attr)

    f32 = lambda a: np.ascontiguousarray(np.asarray(a, dtype=np.float32))
    Wl1, bl1, Wr1, br1 = map(f32, (inputs["Wl1"], inputs["bl1"], inputs["Wr1"], inputs["br1"]))
    We1, att1, bo1 = map(f32, (inputs["We1"], inputs["att1"], inputs["bo1"]))
    Wl2, bl2, Wr2, br2 = map(f32, (inputs["Wl2"], inputs["bl2"], inputs["Wr2"], inputs["br2"]))
    We2, att2, bo2 = map(f32, (inputs["We2"], inputs["att2"], inputs["bo2"]))
    Wl3, bl3, Wr3, br3 = map(f32, (inputs["Wl3"], inputs["bl3"], inputs["Wr3"], inputs["br3"]))
    We3, att3, bo3 = map(f32, (inputs["We3"], inputs["att3"], inputs["bo3"]))
    Wres, bres = map(f32, (inputs["Wres"], inputs["bres"]))
    Wc1, bc1, Wc2, bc2 = map(f32, (inputs["Wc1"], inputs["bc1"], inputs["Wc2"], inputs["bc2"]))

    use_bias = any(np.any(b) for b in (bl1, br1, bo1, bl2, br2, bo2,
                                       bl3, br3, bo3, bres, bc1, bc2))

    key = (T_w, use_bias)
    if key not in _cache:
        _cache[key] = _build_program(T_w, use_bias=use_bias)
    nc = _cache[key]

    common = {
        "x_full": _bf(x),
        "x_f32": x,
        "iotar": np.ascontiguousarray(
            np.tile(np.arange(128, dtype=np.float32).reshape(1, 128), (128, 1))),
        "iotap": np.arange(128, dtype=np.float32).reshape(128, 1),
        "Wl1": _bf(Wl1), "Wl1r": Wl1, "Wr1": Wr1, "Wres": Wres,
        "We1": _bf(We1),
        "att1b": _bf(np.tile(att1.reshape(1, 1024), (128, 1))),
        "Wl2c": _chunks_for_rhs(Wl2), "Wr2c": _chunks_for_rhs(Wr2),
        "We2": _bf(We2), "att2b": _bf(np.tile(att2.reshape(1, 512), (128, 1))),
        "Wl3": Wl3, "Wr3": Wr3, "We3": _bf(We3),
        "att3b": _bf(np.tile(att3.reshape(1, 128), (128, 1))),
        "Wc1": Wc1, "Wc2": np.ascontiguousarray(np.pad(Wc2, ((0, 0), (0, 1)))),
    }
    if use_bias:
        common.update({
            "biasr1": (br1 + bl1).reshape(1, 1024),
            "const1": (bl1 + bo1 + bres).reshape(1, 1024),
            "biasr2": (br2 + bl2).reshape(1, 512),
            "const2b": np.ascontiguousarray(
                np.tile((bl2.reshape(4, 128).mean(0) + bo2).reshape(1, 128),
                        (128, 1))),
            "biasr3": (br3 + bl3).reshape(1, 128),
            "const3b": np.ascontiguousarray(
                np.tile((bl3 + bo3).reshape(1, 128), (128, 1))),
            "bc1": bc1.reshape(1, 64), "bc2": bc2.reshape(1, 3),
        })

    def tilemajor(a, c):
        return np.ascontiguousarray(
            a[c].reshape(W, T_w, 128).transpose(0, 2, 1).reshape(W * 128, T_w))

    in_maps = []
    for c in range(NC_CORES):
        m = dict(common)
        m["x_ownT"] = np.ascontiguousarray(x[node_of[c]].T)
        m["src1i"] = np.concatenate([_idx16_wrap(src1[c, w]) for w in range(W)])
        m["src23i"] = np.concatenate([_idx16_wrap(src23[c, w]) for w in range(W)])
        m["drel"] = tilemajor(drel, c)
        m["drelR"] = _bf(drel[c])
        m["eaT"] = _bf(eaT[c].reshape(W * ED, EPW))
        in_maps.append(m)

    kernel._last_in_maps = in_maps
    res = run_bass_kernel_spmd(nc, in_maps, core_ids=list(range(NC_CORES)),
                               trace=False)
    out = np.zeros((N, 3), np.float32)
    for c in range(NC_CORES):
        o = np.asarray(res.results[c]["out_o"], np.float32)
        out[node_of[c][valid[c]]] = o[valid[c]]
    if _DEBUG:
        kernel._last_results = res.results
        kernel._node_of = node_of
        kernel._valid = valid
    return out
